# revision 1
# baseline (speedup 1.0000x reference)
"""Fused multi-head bilinear attention (softmax over query axis m) on 8 trn2 cores.

Reference computation (b=2, m=n=2048, e=128, k=8, d=16):
    r   = einsum('bmp,kpd->bmkd', x, lambda1) + bias_lambda
    A   = einsum('bmkd,kqd,bnq->kbmn', r, lambda2, y) * d**-0.5
    att = softmax(A, axis=m)
    r2  = einsum('kbmn,bmp,kpd->bnkd', att, x, theta1) + bias_theta
    out = einsum('bnkd,kqd->bnq', r2, theta2)

Sharding: 8 cores = 2 batches x 4 n-quarters (512 wide). Each core computes all 8
heads for its output slice out[b, nq*512:(nq+1)*512, :]; unshard is pure concat.

Per-core pipeline (all heads):
  X^T, Y^T arrive host-pre-transposed; R^T = (X@L1)^T and S^T = (Y@L2)^T with
  heads packed into 32-partition strips (16 used + 16 zero) so K=16 matmuls are
  32-aligned (f32r via rounded copies of X^T/Y^T and the lambdas);
  T = X@theta1 stored [m, (head, 33)]: 16 data cols, 16 zeros, and a ones col at
  32 per head.  Per head: A tiles [m128, 512] = R^T.T @ S^T (f32r), exp on
  ScalarE straight out of PSUM in 2048/1024-wide calls, then U[33, n] +=
  TAUG_k^T @ expA accumulated over m -- row 32 is the softmax denominator for
  free (the ones column).  U rows 0:16 are normalized in place by a reciprocal
  row broadcast across partitions (GpSimd partition_broadcast), giving
  r2^T[(k d), n] strip-packed directly as the final lhsT; out = r2^T.T @
  theta2^T contracts all 128 (k,d) rows at full PE width (zero half-strips on
  both sides keep the sum exact).  U-matmuls are emitted with a lag behind
  A/exp so the PE never head-of-line blocks on the exp of its own group.
"""

import sys

from contextlib import ExitStack

import numpy as np

try:
    import concourse.bass as bass
except ImportError:
    sys.path.append("/opt/trn_rl_repo")
    import concourse.bass as bass
import concourse.tile as tile
from concourse import bacc, mybir
from concourse.bass import ds, ts
from concourse.masks import make_identity

F32 = mybir.dt.float32
F32R = mybir.dt.float32r
EXP = mybir.ActivationFunctionType.Exp

B, M, N, E, K, D = 2, 2048, 2048, 128, 8, 16
NCORES = 8
NSLICE = N // 4          # n columns per core (one batch, quarter of n)
MT = M // 128            # 16 m-tiles
SCALE = float(D) ** -0.5
# m-tile groups for A/exp staging: (start, len) in units of 512-wide m-tiles.
# Groups alternate between two 3-bank PSUM pools; 6 groups per head keeps the
# alternation seamless across head boundaries (no same-pool adjacency).
GROUPS = [(0, 3), (3, 3), (6, 3), (9, 3), (12, 2), (14, 2)]


def _emit(tc: tile.TileContext, ctx: ExitStack, io: dict):
    nc = tc.nc
    xb, ybs, l1g, l2g, t1a, t2t, blg, btb, outb = (
        io["xb"], io["ybs"], io["l1g"], io["l2g"], io["t1a"], io["t2t"],
        io["blg"], io["btb"], io["outb"],
    )

    const = ctx.enter_context(tc.tile_pool(name="const", bufs=1))
    persist = ctx.enter_context(tc.tile_pool(name="persist", bufs=1))
    expa3_pool = ctx.enter_context(tc.tile_pool(name="expa3", bufs=3))
    expa2_pool = ctx.enter_context(tc.tile_pool(name="expa2", bufs=3))
    den_pool = ctx.enter_context(tc.tile_pool(name="den", bufs=4))
    out_pool = ctx.enter_context(tc.tile_pool(name="outp", bufs=2))
    ps_pa = ctx.enter_context(tc.tile_pool(name="ps_pa", bufs=1, space="PSUM"))
    ps_pb = ctx.enter_context(tc.tile_pool(name="ps_pb", bufs=1, space="PSUM"))
    ps_u = ctx.enter_context(tc.tile_pool(name="ps_u", bufs=2, space="PSUM"))

    pp = [0]

    def ping(shape):
        # strict global alternation between the two 3-bank PSUM staging pools
        pp[0] ^= 1
        pool, tag = (ps_pa, "pa") if pp[0] else (ps_pb, "pb")
        return pool.tile(shape, F32, tag=tag, name="pro%s" % tag)

    # ---- parameter loads -------------------------------------------------
    L1 = const.tile([128, 2, 128], F32)   # strip-packed lambda1 per head-group
    L2 = const.tile([128, 2, 128], F32)
    T1A = const.tile([128, 128], F32)     # theta1 packed (k d)
    T2T = const.tile([128, 2, 128], F32)  # strip-packed theta2^T per group
    BLG = const.tile([128, 2], F32)       # strip-packed bias_lambda
    BTC = const.tile([128, 2], F32)       # strip-packed bias_theta
    XT = persist.tile([128, M], F32, name="XT")       # [e, m]
    YT = persist.tile([128, NSLICE], F32, name="YT")  # [e, n]
    nc.sync.dma_start(YT[:], ybs)
    nc.sync.dma_start(XT[:, 0:512], xb[:, 0:512])
    for g in range(2):
        nc.sync.dma_start(L1[:, g, :], l1g[g])
        nc.sync.dma_start(L2[:, g, :], l2g[g])
    nc.sync.dma_start(T1A[:], t1a)
    for g in range(2):
        nc.sync.dma_start(T2T[:, g, :], t2t[g])
    nc.sync.dma_start(BLG[:], blg)
    nc.sync.dma_start(BTC[:], btb)

    # ---- persistent intermediates ---------------------------------------
    # X^T / Y^T arrive pre-transposed from the host (layout prep), plain f32;
    # the projection matmuls consuming them run fp32 and their PSUM
    # evacuations round into f32r tiles.
    ident = const.tile([128, 128], F32)
    make_identity(nc, ident[:])
    # dummy transposes keep the PE busy (and ramp its p-state) while the
    # first input DMAs are still in flight
    for _w in range(28):
        wp = ping([128, 128])
        nc.tensor.transpose(wp[:], ident[:], ident[:])
    XTR = persist.tile([128, M], F32R)     # f32r copies feed the projections
    YTR = persist.tile([128, NSLICE], F32R)
    RT = persist.tile([128, 2, M], F32R)       # R^T strips [32h+j, g, m]
    ST = persist.tile([128, 2, NSLICE], F32R)  # S^T strips
    # per head 33 lhsT columns: 16 of X@theta1, 16 zeros, ones at 32 so the
    # denominator lands on a 32-aligned U row
    TAUG = persist.tile([128, MT, K * 33], F32R)
    R2TG = persist.tile([128, 2, NSLICE], F32)  # strip-packed [(h d) g n]
    ONES = const.tile([128, MT * K], F32)
    nc.gpsimd.memset(ONES[:], 1.0)
    ZEROS = const.tile([128, MT * K * 16], F32)
    nc.gpsimd.memset(ZEROS[:], 0.0)
    nc.gpsimd.memset(R2TG[:], 0.0)
    nc.vector.tensor_copy(
        TAUG[:].rearrange("p mt (k s) -> p mt k s", k=K)[:, :, :, 32:33],
        ONES[:].rearrange("p (mt k) -> p mt k", k=K)[:, :, :, None])
    nc.vector.tensor_copy(
        TAUG[:].rearrange("p mt (k s) -> p mt k s", k=K)[:, :, :, 16:32],
        ZEROS[:].rearrange("p (mt k d) -> p mt k d", k=K, d=16))
    L1R = const.tile([128, 2, 128], F32R)
    L2R = const.tile([128, 2, 128], F32R)
    nc.vector.tensor_copy(L1R[:], L1[:])
    nc.vector.tensor_copy(L2R[:], L2[:])

    def y_block():
        ps = ping([128, NSLICE])
        nc.tensor.matmul(ps[:], lhsT=L2[:, 0, :], rhs=YT[:], start=True, stop=True)
        nc.vector.tensor_copy(ST[:, 0, :], ps[:])
        nc.vector.tensor_copy(YTR[:], YT[:])

    def q4_block(q4):
        if q4 > 0:
            nc.sync.dma_start(XT[:, ts(q4, 512)], xb[:, ts(q4, 512)])
        ps = ping([128, 512])
        nc.tensor.matmul(ps[:], lhsT=L1[:, 0, :], rhs=XT[:, ts(q4, 512)],
                         start=True, stop=True)
        nc.vector.tensor_scalar_add(RT[:, 0, ts(q4, 512)], ps[:], BLG[:, 0:1])
        nc.vector.tensor_copy(XTR[:, ts(q4, 512)], XT[:, ts(q4, 512)])
        for j in range(4):
            mt = q4 * 4 + j
            ps = ping([128, 128])
            nc.tensor.matmul(ps[:], lhsT=XT[:, ts(mt, 128)], rhs=T1A[:],
                             start=True, stop=True)
            nc.vector.tensor_copy(
                TAUG[:, mt, :].rearrange("p (k s) -> p k s", k=K)[:, :, 0:16],
                ps[:].rearrange("p (k d) -> p k d", k=K))

    # U accumulators are [33, n]: rows 0-15 numerator, row 32 denominator

    def rs_g1_block():
        # group-1 projections in two wide tiles: back-to-back matmuls with a
        # single evacuation each, so the pipeline is not head-of-line blocked
        # by a slot->evac->slot chain when this pops mid-stream
        ps = ping([128, 1536])
        for c in range(3):
            nc.tensor.matmul(ps[:, ts(c, 512)], lhsT=L1R[:, 1, :],
                             rhs=XTR[:, ts(c, 512)], start=True, stop=True)
        nc.vector.tensor_scalar_add(RT[:, 1, 0:1536], ps[:], BLG[:, 1:2])
        ps = ping([128, 1024])
        nc.tensor.matmul(ps[:, 0:512], lhsT=L1R[:, 1, :], rhs=XTR[:, ts(3, 512)],
                         start=True, stop=True)
        nc.tensor.matmul(ps[:, 512:1024], lhsT=L2R[:, 1, :], rhs=YTR[:],
                         start=True, stop=True)
        nc.vector.tensor_scalar_add(RT[:, 1, ts(3, 512)], ps[:, 0:512],
                                    BLG[:, 1:2])
        nc.vector.tensor_copy(ST[:, 1, :], ps[:, 512:1024])

    # ---- head pipeline: U-matmuls emitted with a lag ---------------------
    LAG = 3
    pending = []

    def flush(limit):
        while len(pending) > limit:
            pending.pop(0)()

    def mk_ubatch(U, k, mst, glen, expa):
        def emit():
            for j in range(glen):
                mt = mst + j
                nc.tensor.matmul(
                    U[:], lhsT=TAUG[:, mt, ds(33 * k, 33)],
                    rhs=expa[:, ts(j, 512)],
                    start=(mt == 0), stop=(mt == MT - 1))
        return emit

    def mk_finalize(U, k, split=False):
        g, h = divmod(k, 4)
        strip = 32 * h

        def emit():
            den = den_pool.tile([1, NSLICE], F32, tag="den", name="den")
            nc.vector.reciprocal(den[:], U[32:33, :])
            rb = den_pool.tile([16, NSLICE], F32, tag="rb", name="rb")
            nc.gpsimd.partition_broadcast(rb[:], den[:])
            # for the last head, normalize chunk-by-chunk so the output
            # matmuls can start on chunk 0 before the whole row is done
            chunks = [ts(c, 128) for c in range(NSLICE // 128)] if split \
                else [slice(0, NSLICE)]
            for sl in chunks:
                nc.vector.tensor_mul(
                    R2TG[strip:strip + 16, g, sl], U[0:16, sl], rb[:, sl])
                nc.vector.tensor_scalar_add(
                    R2TG[strip:strip + 16, g, sl],
                    R2TG[strip:strip + 16, g, sl],
                    BTC[strip:strip + 16, g:g + 1])
        return emit

    heads_state = {}

    def head_group(k, gi):
        g, h = divmod(k, 4)
        strip = 32 * h
        if gi == 0:
            heads_state[k] = ps_u.tile([33, NSLICE], F32, tag="u", name="U")
        U = heads_state[k]
        mst, glen = GROUPS[gi]
        aps = ping([128, 512 * glen])
        for j in range(glen):
            mt = mst + j
            nc.tensor.matmul(
                aps[:, ts(j, 512)],
                lhsT=RT[strip:strip + 16, g, ds(mt * 128, 128)],
                rhs=ST[strip:strip + 16, g, :],
                start=True, stop=True, tile_position=(strip, 0))
        epool = expa3_pool if glen == 3 else expa2_pool
        expa = epool.tile([128, 512 * glen], F32R, tag="e%d" % glen, name="expa")
        nc.scalar.activation(expa[:], aps[:], EXP, scale=SCALE)
        pending.append(mk_ubatch(U, k, mst, glen, expa))
        flush(LAG)
        if gi == len(GROUPS) - 1:
            pending.append(mk_finalize(U, k, split=(k == K - 1)))

    # prologue interleaved with heads 0-1 (group gi needs RT chunks <= its mts)
    y_block()
    q4_block(0)
    head_group(0, 0)
    head_group(1, 0)
    q4_block(1)
    head_group(0, 1)
    head_group(1, 1)
    q4_block(2)
    head_group(0, 2)
    head_group(1, 2)
    head_group(0, 3)
    head_group(1, 3)
    q4_block(3)
    head_group(0, 4)
    head_group(1, 4)
    head_group(0, 5)
    head_group(1, 5)
    pending.insert(0, rs_g1_block)
    for k in range(2, K):
        for gi in range(len(GROUPS)):
            head_group(k, gi)
    flush(0)

    # ---- output: out[n, q] = (r2 + bias_theta) @ theta2^T ---------------
    # r2 and theta2^T are strip-packed with zeros in the unused half-strips,
    # so accumulating both groups' full-K matmuls gives the exact sum over kd.
    OB = out_pool.tile([128, NSLICE // 128, 128], F32, tag="ob")
    for ch in range(NSLICE // 128):
        op = ping([128, 128])
        for g in range(2):
            nc.tensor.matmul(op[:], lhsT=R2TG[:, g, ts(ch, 128)], rhs=T2T[:, g, :],
                             start=(g == 0), stop=(g == 1))
        nc.vector.tensor_copy(OB[:, ch, :], op[:])
        if ch == 1:
            nc.sync.dma_start(
                outb[0:256, :].rearrange("(c p) q -> p c q", p=128), OB[:, 0:2, :])
    nc.sync.dma_start(
        outb[256:512, :].rearrange("(c p) q -> p c q", p=128), OB[:, 2:4, :])


_CACHE = {}


def build():
    if "nc" in _CACHE:
        return _CACHE["nc"]
    nc = bacc.Bacc("TRN2", target_bir_lowering=False, debug=False,
                   num_devices=NCORES)
    io = {
        "xb": nc.dram_tensor("xb", [E, M], F32, kind="ExternalInput").ap(),
        "ybs": nc.dram_tensor("ybs", [E, NSLICE], F32, kind="ExternalInput").ap(),
        "l1g": nc.dram_tensor("l1g", [2, E, 128], F32, kind="ExternalInput").ap(),
        "l2g": nc.dram_tensor("l2g", [2, E, 128], F32, kind="ExternalInput").ap(),
        "t1a": nc.dram_tensor("t1a", [E, 128], F32, kind="ExternalInput").ap(),
        "t2t": nc.dram_tensor("t2t", [2, 128, E], F32, kind="ExternalInput").ap(),
        "blg": nc.dram_tensor("blg", [128, 2], F32, kind="ExternalInput").ap(),
        "btb": nc.dram_tensor("btb", [128, 2], F32, kind="ExternalInput").ap(),
        "outb": nc.dram_tensor("outb", [NSLICE, E], F32, kind="ExternalOutput").ap(),
    }
    with tile.TileContext(nc) as tc:
        with ExitStack() as ctx:
            _emit(tc, ctx, io)
    nc.compile()
    _CACHE["nc"] = nc
    return nc


def make_in_maps(x, y, lambda1, lambda2, theta1, theta2, bias_lambda, bias_theta):
    f = np.float32
    l1g = np.zeros((2, E, 128), f)
    l2g = np.zeros((2, E, 128), f)
    t2t = np.zeros((2, 128, E), f)
    blg = np.zeros((128, 2), f)
    btb = np.zeros((128, 2), f)
    for g in range(2):
        for h in range(4):
            l1g[g, :, 32 * h:32 * h + 16] = lambda1[4 * g + h]
            l2g[g, :, 32 * h:32 * h + 16] = lambda2[4 * g + h]
            t2t[g, 32 * h:32 * h + 16, :] = theta2[4 * g + h].T
            blg[32 * h:32 * h + 16, g] = bias_lambda[4 * g + h]
            btb[32 * h:32 * h + 16, g] = bias_theta[4 * g + h]
    t1a = np.ascontiguousarray(theta1.transpose(1, 0, 2).reshape(E, K * D))
    xts = [np.ascontiguousarray(np.asarray(x[b], dtype=f).T) for b in range(B)]
    maps = []
    for c in range(NCORES):
        b, q = divmod(c, 4)
        maps.append({
            "xb": xts[b],
            "ybs": np.ascontiguousarray(
                np.asarray(y[b, q * NSLICE:(q + 1) * NSLICE], dtype=f).T),
            "l1g": l1g, "l2g": l2g, "t1a": t1a, "t2t": t2t,
            "blg": blg, "btb": btb,
        })
    return maps


def kernel(x, y, lambda1, lambda2, theta1, theta2, bias_lambda, bias_theta):
    from concourse.bass_utils import run_bass_kernel_spmd
    nc = build()
    maps = make_in_maps(x, y, lambda1, lambda2, theta1, theta2,
                        bias_lambda, bias_theta)
    res = run_bass_kernel_spmd(nc, maps, list(range(NCORES)))
    out = np.empty((B, N, E), np.float32)
    for c in range(NCORES):
        b, q = divmod(c, 4)
        out[b, q * NSLICE:(q + 1) * NSLICE] = res.results[c]["outb"]
    return out



# revision 9
# speedup vs baseline: 1.4023x; 1.4023x over previous
"""Fused multi-head bilinear attention (softmax over query axis m) on 8 trn2 cores.

Reference computation (b=2, m=n=2048, e=128, k=8, d=16):
    r   = einsum('bmp,kpd->bmkd', x, lambda1) + bias_lambda
    A   = einsum('bmkd,kqd,bnq->kbmn', r, lambda2, y) * d**-0.5
    att = softmax(A, axis=m)
    r2  = einsum('kbmn,bmp,kpd->bnkd', att, x, theta1) + bias_theta
    out = einsum('bnkd,kqd->bnq', r2, theta2)

Sharding: 8 cores = 2 batches x 4 n-quarters (512 wide). Each core computes all 8
heads for its output slice out[b, nq*512:(nq+1)*512, :]; unshard is pure concat.

Math simplifications (exact):
  - bias_lambda adds a per-(k,n) constant to every logit column; softmax over m
    is invariant to it, so it is dropped entirely.
  - bias_theta folds into a constant output bias c_q = sum_kd bias_theta*theta2,
    precomputed on host and added during the output-PSUM evacuation.

Per-core pipeline (all heads):
  R^T/S^T strips (f32r, 2 head-groups at 32-aligned strips) and T = X@theta1
  (bf16); inputs DMA straight into f32r/bf16 tiles (host supplies both).
  A tiles [m128, n512] = R^T.T @ S^T (f32r) in 1024-wide groups staged through
  THREE 2-bank PSUM pools (depth-3 software pipeline), exp'd alternately by
  ScalarE (true exp -> bf16) and VectorE (Schraudolph bit-trick exp: i16 =
  round(A*scale*128*log2e + (16256-C)) bitcast to bf16, C=7.4) so both engines
  share the 65536-row exp load; projection evacuations ride ScalarE.
  The U matmul is FLIPPED: U^T[n-chunk 128, 17] += expa_chunk^T @
  TAUG[m, (16 cols of T | ones)] in bf16 -- 17-row matmuls instead of 512-row
  f32r streams (65536 -> 8704 PE rows); the ones column makes output column 16
  the softmax denominator.  Chunk chains are emitted contiguously per head
  (one open PSUM accumulation group per bank at a time), one batch per head,
  lagged one head behind the A/exp stream.  Finalize = one reciprocal + one
  broadcast tensor_tensor multiply into R2U[n, (k d)] bf16; 4 PE transposes
  give r2^T[(k d), n] and one full-width bf16 matmul against theta2^T (tight
  (k,d) x q packing) yields out^T[q, n], evacuated with the c_q bias and DMA'd
  as outb[E, NSLICE] (host transposes back).
"""

import sys

from contextlib import ExitStack

import numpy as np

try:
    import concourse.bass as bass
except ImportError:
    sys.path.append("/opt/trn_rl_repo")
    import concourse.bass as bass
import concourse.tile as tile
from concourse import bacc, mybir
from concourse.bass import ds, ts
from concourse.masks import make_identity

F32 = mybir.dt.float32
F32R = mybir.dt.float32r
BF16 = mybir.dt.bfloat16
I16 = mybir.dt.int16
EXP = mybir.ActivationFunctionType.Exp
MULT = mybir.AluOpType.mult
ADD = mybir.AluOpType.add

B, M, N, E, K, D = 2, 2048, 2048, 128, 8, 16
NCORES = 8
NSLICE = N // 4          # n columns per core (one batch, quarter of n)
MT = M // 128            # 16 m-tiles
SCALE = float(D) ** -0.5
LOG2E = 1.4426950408889634
SCH_C = 7.4              # Schraudolph bias tweak (exponent lsb units)
SCH_A = SCALE * 128.0 * LOG2E
SCH_B = 127.0 * 128.0 - SCH_C
NG = 8                   # A/exp groups per head, 2 m-tiles (1024 wide) each
# exp engine per (head, group): 'D' = DVE Schraudolph, 'A' = ScalarE true exp
EXP_SCHED = [list("DADADADA") if k % 2 == 0 else list("ADADADAD")
             for k in range(K)]


def _emit(tc: tile.TileContext, ctx: ExitStack, io: dict):
    nc = tc.nc
    xtr, xtb, ybr, l1r, l2r, t1ab, t2tb, cq, outb = (
        io["xtr"], io["xtb"], io["ybr"], io["l1r"], io["l2r"], io["t1ab"],
        io["t2tb"], io["cq"], io["outb"],
    )

    const = ctx.enter_context(tc.tile_pool(name="const", bufs=1))
    persist = ctx.enter_context(tc.tile_pool(name="persist", bufs=1))
    # a head's expa tiles stay alive until its U batch runs at the END of the
    # next head's groups; 3 head-generations of 8 tiles make reuse safe
    expa_pool = ctx.enter_context(tc.tile_pool(name="expa", bufs=24))
    fin_pool = ctx.enter_context(tc.tile_pool(name="fin", bufs=4))
    ps_pa = ctx.enter_context(tc.tile_pool(name="ps_pa", bufs=1, space="PSUM"))
    ps_pb = ctx.enter_context(tc.tile_pool(name="ps_pb", bufs=1, space="PSUM"))
    ps_pc = ctx.enter_context(tc.tile_pool(name="ps_pc", bufs=1, space="PSUM"))
    ps_u = ctx.enter_context(tc.tile_pool(name="ps_u", bufs=2, space="PSUM"))

    pp = [0]
    pools = [(ps_pa, "pa"), (ps_pb, "pb"), (ps_pc, "pc")]

    def ping(shape, dtype=F32):
        # strict round-robin through the three 2-bank PSUM staging pools
        pp[0] = (pp[0] + 1) % 3
        pool, tag = pools[pp[0]]
        return pool.tile(shape, dtype, tag=tag, name="pro%s" % tag)

    # ---- parameter + input DMA (straight into compute dtypes) ------------
    L1R = const.tile([128, 2, 128], F32R)  # strip-packed lambda1 per group
    L2R = const.tile([128, 2, 128], F32R)
    T1AB = const.tile([128, 128], BF16)    # theta1 packed (k d), bf16
    T2TB = const.tile([128, 128], BF16)    # theta2^T tight (k d) x q, bf16
    CQ = const.tile([128, 1], F32)         # host-folded bias_theta @ theta2
    XTR = persist.tile([128, M], F32R, name="XTR")    # [e, m] f32r
    XTB = persist.tile([128, M], BF16, name="XTB")    # [e, m] bf16
    YTR = persist.tile([128, NSLICE], F32R, name="YTR")
    nc.sync.dma_start(YTR[:], ybr)
    nc.sync.dma_start(XTR[:, 0:512], xtr[:, 0:512])
    nc.sync.dma_start(XTB[:, 0:512], xtb[:, 0:512])
    for g in range(2):
        nc.sync.dma_start(L1R[:, g, :], l1r[g])
        nc.sync.dma_start(L2R[:, g, :], l2r[g])
    nc.sync.dma_start(T1AB[:], t1ab)
    nc.sync.dma_start(T2TB[:], t2tb)
    nc.sync.dma_start(CQ[:], cq)

    # ---- persistent intermediates ---------------------------------------
    identb = const.tile([128, 128], BF16)
    make_identity(nc, identb[:])
    # dummy transposes keep the PE busy (and ramp its p-state) while the
    # first input DMAs are still in flight
    for _w in range(24):
        wp = ping([128, 128], BF16)
        nc.tensor.transpose(wp[:], identb[:], identb[:])
    RT = persist.tile([128, 2, M], F32R)       # R^T strips [32h+j, g, m]
    ST = persist.tile([128, 2, NSLICE], F32R)  # S^T strips
    # TAUG[:, mt, k, 0:16] = (X@theta1)^T-tile for head k (bf16); col 16 = 1.0
    # so the U matmul's 17th output column is the softmax denominator.
    TAUG = persist.tile([128, MT, K, 17], BF16)
    R2U = persist.tile([128, 4, K, 16], BF16)  # normalized U^T chunks
    R2T = persist.tile([128, NSLICE], BF16)    # r2^T [(k d) tight, n]
    nc.gpsimd.memset(TAUG[:, :, :, 16:17], 1.0)

    def y_block():
        ps = ping([128, 2, NSLICE])
        for g in range(2):
            nc.tensor.matmul(ps[:, g, :], lhsT=L2R[:, g, :], rhs=YTR[:],
                             start=True, stop=True)
        nc.scalar.copy(ST[:], ps[:])

    def q4_block(q4):
        # R^T group-0 projection + T tiles for m-tiles 4q..4q+3
        if q4 > 0:
            nc.sync.dma_start(XTR[:, ts(q4, 512)], xtr[:, ts(q4, 512)])
            nc.sync.dma_start(XTB[:, ts(q4, 512)], xtb[:, ts(q4, 512)])
        ps = ping([128, 512])
        nc.tensor.matmul(ps[:], lhsT=L1R[:, 0, :], rhs=XTR[:, ts(q4, 512)],
                         start=True, stop=True)
        nc.scalar.copy(RT[:, 0, ts(q4, 512)], ps[:])
        pt = ping([128, 4, K, 16])
        for j in range(4):
            mt = q4 * 4 + j
            nc.tensor.matmul(pt[:, j].rearrange("p k d -> p (k d)"),
                             lhsT=XTB[:, ts(mt, 128)], rhs=T1AB[:],
                             start=True, stop=True)
        nc.scalar.copy(TAUG[:, ds(q4 * 4, 4), :, 0:16], pt[:])

    def rs_g1_block():
        # group-1 R^T projection in two 1024-wide chunks
        for half in range(2):
            ps = ping([128, 1024])
            for c in range(2):
                nc.tensor.matmul(ps[:, ts(c, 512)], lhsT=L1R[:, 1, :],
                                 rhs=XTR[:, ts(2 * half + c, 512)],
                                 start=True, stop=True)
            nc.scalar.copy(RT[:, 1, ts(half, 1024)], ps[:])

    # ---- head pipeline: per-head U batch emitted one head late -----------
    pending = []

    def flush(limit):
        while len(pending) > limit:
            pending.pop(0)()

    def mk_ubatch(U, k, expas):
        # one contiguous accumulation chain per n-chunk: the executor supports
        # only one open PSUM accumulation group per bank, so chunk chains must
        # not interleave
        def emit():
            for c in range(4):
                for expa, mst, glen in expas:
                    for j in range(glen):
                        mt = mst + j
                        nc.tensor.matmul(
                            U[:, c, :],
                            lhsT=expa[:, ds(j * 512 + c * 128, 128)],
                            rhs=TAUG[:, mt, k, :],
                            start=(mt == 0), stop=(mt == MT - 1))
        return emit

    def mk_finalize(U, k):
        def emit():
            rec = fin_pool.tile([128, 4], F32, tag="rec", name="rec")
            nc.vector.reciprocal(rec[:], U[:, :, 16])
            nc.vector.tensor_tensor(
                R2U[:, :, k, :], U[:, :, 0:16],
                rec[:, :, None].broadcast_to((128, 4, 16)), MULT)
        return emit

    heads_state = {}

    def head_group(k, gi):
        g, h = divmod(k, 4)
        strip = 32 * h
        if gi == 0:
            heads_state[k] = (
                ps_u.tile([128, 4, 17], F32, tag="u", name="U"), [])
        U, expas = heads_state[k]
        mst, glen = 2 * gi, 2
        aps = ping([128, 512 * glen])
        for j in range(glen):
            mt = mst + j
            nc.tensor.matmul(
                aps[:, ts(j, 512)],
                lhsT=RT[strip:strip + 16, g, ds(mt * 128, 128)],
                rhs=ST[strip:strip + 16, g, :],
                start=True, stop=True, tile_position=(strip, 0))
        expa = expa_pool.tile([128, 512 * glen], BF16, tag="e", name="expa")
        if EXP_SCHED[k][gi] == "A":
            nc.scalar.activation(expa[:], aps[:], EXP, scale=SCALE)
        else:
            nc.vector.tensor_scalar(
                expa[:].bitcast(I16), aps[:], SCH_A, SCH_B, MULT, ADD)
        expas.append((expa, mst, glen))
        if gi == NG - 1:
            ub = mk_ubatch(U, k, expas)
            fin = mk_finalize(U, k)
            pending.append(lambda: (ub(), fin()))
            flush(1)

    # prologue interleaved with heads 0-1 (group gi needs RT chunks <= its mts)
    y_block()
    q4_block(0)
    head_group(0, 0)
    head_group(1, 0)
    q4_block(1)
    head_group(0, 1)
    head_group(1, 1)
    q4_block(2)
    head_group(0, 2)
    head_group(1, 2)
    head_group(0, 3)
    head_group(1, 3)
    q4_block(3)
    for gi in range(4, NG):
        head_group(0, gi)
        head_group(1, gi)
    pending.insert(0, rs_g1_block)
    for k in range(2, K):
        for gi in range(NG):
            head_group(k, gi)
    flush(0)

    # ---- output: out^T[q, n] = theta2^T(tight).T @ r2^T + c_q ------------
    for c in range(4):
        tp = ps_u.tile([128, 128], BF16, tag="u", name="tp")
        nc.tensor.transpose(
            tp[:], R2U[:, c, :, :].rearrange("p k d -> p (k d)"), identb[:])
        nc.vector.tensor_copy(R2T[:, ts(c, 128)], tp[:])
    ot = ps_u.tile([128, NSLICE], F32, tag="u", name="ot")
    nc.tensor.matmul(ot[:], lhsT=T2TB[:], rhs=R2T[:], start=True, stop=True)
    OB = fin_pool.tile([128, NSLICE], F32, tag="ob")
    nc.vector.tensor_scalar_add(OB[:, 0:256], ot[:, 0:256], CQ[:])
    nc.sync.dma_start(outb[:, 0:256], OB[:, 0:256])
    nc.vector.tensor_scalar_add(OB[:, 256:512], ot[:, 256:512], CQ[:])
    nc.sync.dma_start(outb[:, 256:512], OB[:, 256:512])


_CACHE = {}


def build():
    if "nc" in _CACHE:
        return _CACHE["nc"]
    nc = bacc.Bacc("TRN2", target_bir_lowering=False, debug=False,
                   num_devices=NCORES)
    io = {
        "xtr": nc.dram_tensor("xtr", [E, M], F32R, kind="ExternalInput").ap(),
        "xtb": nc.dram_tensor("xtb", [E, M], BF16, kind="ExternalInput").ap(),
        "ybr": nc.dram_tensor("ybr", [E, NSLICE], F32R, kind="ExternalInput").ap(),
        "l1r": nc.dram_tensor("l1r", [2, E, 128], F32R, kind="ExternalInput").ap(),
        "l2r": nc.dram_tensor("l2r", [2, E, 128], F32R, kind="ExternalInput").ap(),
        "t1ab": nc.dram_tensor("t1ab", [E, K * D], BF16, kind="ExternalInput").ap(),
        "t2tb": nc.dram_tensor("t2tb", [K * D, E], BF16, kind="ExternalInput").ap(),
        "cq": nc.dram_tensor("cq", [E, 1], F32, kind="ExternalInput").ap(),
        "outb": nc.dram_tensor("outb", [E, NSLICE], F32, kind="ExternalOutput").ap(),
    }
    with tile.TileContext(nc) as tc:
        with ExitStack() as ctx:
            _emit(tc, ctx, io)
    nc.compile()
    _CACHE["nc"] = nc
    return nc


def make_in_maps(x, y, lambda1, lambda2, theta1, theta2, bias_lambda, bias_theta):
    import ml_dtypes
    f = np.float32
    bf = ml_dtypes.bfloat16
    l1r = np.zeros((2, E, 128), f)
    l2r = np.zeros((2, E, 128), f)
    for g in range(2):
        for h in range(4):
            l1r[g, :, 32 * h:32 * h + 16] = lambda1[4 * g + h]
            l2r[g, :, 32 * h:32 * h + 16] = lambda2[4 * g + h]
    # theta1 packed [e, (k d)]; theta2^T tight [(k d), q]; bias_theta folded
    t1ab = np.ascontiguousarray(
        theta1.transpose(1, 0, 2).reshape(E, K * D)).astype(bf)
    t2tb = np.ascontiguousarray(
        theta2.transpose(0, 2, 1).reshape(K * D, E)).astype(bf)
    cq = np.einsum("kd,kqd->q", np.asarray(bias_theta, f),
                   np.asarray(theta2, f)).reshape(E, 1).astype(f)
    xts = [np.ascontiguousarray(np.asarray(x[b], dtype=f).T) for b in range(B)]
    xtbs = [xt.astype(bf) for xt in xts]
    maps = []
    for c in range(NCORES):
        b, q = divmod(c, 4)
        maps.append({
            "xtr": xts[b],
            "xtb": xtbs[b],
            "ybr": np.ascontiguousarray(
                np.asarray(y[b, q * NSLICE:(q + 1) * NSLICE], dtype=f).T),
            "l1r": l1r, "l2r": l2r, "t1ab": t1ab, "t2tb": t2tb, "cq": cq,
        })
    return maps


def kernel(x, y, lambda1, lambda2, theta1, theta2, bias_lambda, bias_theta):
    from concourse.bass_utils import run_bass_kernel_spmd
    nc = build()
    maps = make_in_maps(x, y, lambda1, lambda2, theta1, theta2,
                        bias_lambda, bias_theta)
    res = run_bass_kernel_spmd(nc, maps, list(range(NCORES)))
    out = np.empty((B, N, E), np.float32)
    for c in range(NCORES):
        b, q = divmod(c, 4)
        out[b, q * NSLICE:(q + 1) * NSLICE] = res.results[c]["outb"].T
    return out


# revision 11
# speedup vs baseline: 1.4251x; 1.0162x over previous
"""Fused multi-head bilinear attention (softmax over query axis m) on 8 trn2 cores.

Reference computation (b=2, m=n=2048, e=128, k=8, d=16):
    r   = einsum('bmp,kpd->bmkd', x, lambda1) + bias_lambda
    A   = einsum('bmkd,kqd,bnq->kbmn', r, lambda2, y) * d**-0.5
    att = softmax(A, axis=m)
    r2  = einsum('kbmn,bmp,kpd->bnkd', att, x, theta1) + bias_theta
    out = einsum('bnkd,kqd->bnq', r2, theta2)

Sharding: 8 cores = 2 batches x 4 n-quarters (512 wide). Each core computes all 8
heads for its output slice out[b, nq*512:(nq+1)*512, :]; unshard is pure concat.

Math simplifications (exact):
  - bias_lambda adds a per-(k,n) constant to every logit column; softmax over m
    is invariant to it, so it is dropped entirely.
  - bias_theta folds into a constant output bias c_q = sum_kd bias_theta*theta2,
    precomputed on host and added during the output-PSUM evacuation.

Per-core pipeline (all heads):
  R^T/S^T strips (f32r, 2 head-groups at 32-aligned strips) and T = X@theta1
  (bf16); inputs DMA straight into f32r/bf16 tiles (host supplies both).
  A tiles [m128, n512] = R^T.T @ S^T (f32r) in 1024-wide groups staged through
  THREE 2-bank PSUM pools (depth-3 software pipeline), exp'd alternately by
  ScalarE (true exp -> bf16) and VectorE (Schraudolph bit-trick exp: i16 =
  round(A*scale*128*log2e + (16256-C)) bitcast to bf16, C=7.4) so both engines
  share the 65536-row exp load; projection evacuations ride ScalarE.
  The U matmul is FLIPPED: U^T[n-chunk 128, 17] += expa_chunk^T @
  TAUG[m, (16 cols of T | ones)] in bf16 -- 17-row matmuls instead of 512-row
  f32r streams (65536 -> 8704 PE rows); the ones column makes output column 16
  the softmax denominator.  Chunk chains are emitted contiguously per head
  (one open PSUM accumulation group per bank at a time), one batch per head,
  lagged one head behind the A/exp stream.  Finalize = one reciprocal + one
  broadcast tensor_tensor multiply into R2U[n, (k d)] bf16; 4 PE transposes
  give r2^T[(k d), n] and one full-width bf16 matmul against theta2^T (tight
  (k,d) x q packing) yields out^T[q, n], evacuated with the c_q bias and DMA'd
  as outb[E, NSLICE] (host transposes back).
"""

import sys

from contextlib import ExitStack

import numpy as np

try:
    import concourse.bass as bass
except ImportError:
    sys.path.append("/opt/trn_rl_repo")
    import concourse.bass as bass
import concourse.tile as tile
from concourse import bacc, mybir
from concourse.bass import ds, ts
from concourse.masks import make_identity

F32 = mybir.dt.float32
F32R = mybir.dt.float32r
BF16 = mybir.dt.bfloat16
I16 = mybir.dt.int16
EXP = mybir.ActivationFunctionType.Exp
MULT = mybir.AluOpType.mult
ADD = mybir.AluOpType.add

B, M, N, E, K, D = 2, 2048, 2048, 128, 8, 16
NCORES = 8
NSLICE = N // 4          # n columns per core (one batch, quarter of n)
MT = M // 128            # 16 m-tiles
SCALE = float(D) ** -0.5
LOG2E = 1.4426950408889634
SCH_C = 7.4              # Schraudolph bias tweak (exponent lsb units)
SCH_A = SCALE * 128.0 * LOG2E
SCH_B = 127.0 * 128.0 - SCH_C
NG = 8                   # A/exp groups per head, 2 m-tiles (1024 wide) each
# exp engine per (head, group): 'D' = DVE Schraudolph, 'A' = ScalarE true exp
EXP_SCHED = [list("DADADADA") if k % 2 == 0 else list("ADADADAD")
             for k in range(K)]


def _emit(tc: tile.TileContext, ctx: ExitStack, io: dict):
    nc = tc.nc
    xtr, xtb, ybr, l1r, l2r, t1ab, t2tb, cq, outb = (
        io["xtr"], io["xtb"], io["ybr"], io["l1r"], io["l2r"], io["t1ab"],
        io["t2tb"], io["cq"], io["outb"],
    )

    const = ctx.enter_context(tc.tile_pool(name="const", bufs=1))
    persist = ctx.enter_context(tc.tile_pool(name="persist", bufs=1))
    # a head's expa tiles stay alive until its U batch runs at the END of the
    # next head's groups; 3 head-generations of 8 tiles make reuse safe
    expa_pool = ctx.enter_context(tc.tile_pool(name="expa", bufs=24))
    fin_pool = ctx.enter_context(tc.tile_pool(name="fin", bufs=4))
    ps_pa = ctx.enter_context(tc.tile_pool(name="ps_pa", bufs=1, space="PSUM"))
    ps_pb = ctx.enter_context(tc.tile_pool(name="ps_pb", bufs=1, space="PSUM"))
    ps_pc = ctx.enter_context(tc.tile_pool(name="ps_pc", bufs=1, space="PSUM"))
    ps_u = ctx.enter_context(tc.tile_pool(name="ps_u", bufs=2, space="PSUM"))

    pp = [0]
    pools = [(ps_pa, "pa"), (ps_pb, "pb"), (ps_pc, "pc")]

    def ping(shape, dtype=F32):
        # strict round-robin through the three 2-bank PSUM staging pools
        pp[0] = (pp[0] + 1) % 3
        pool, tag = pools[pp[0]]
        return pool.tile(shape, dtype, tag=tag, name="pro%s" % tag)

    # ---- parameter + input DMA (straight into compute dtypes) ------------
    L1R = const.tile([128, 2, 128], F32R)  # strip-packed lambda1 per group
    L2R = const.tile([128, 2, 128], F32R)
    T1AB = const.tile([128, 128], BF16)    # theta1 packed (k d), bf16
    T2TB = const.tile([128, 128], BF16)    # theta2^T tight (k d) x q, bf16
    CQ = const.tile([128, 1], F32)         # host-folded bias_theta @ theta2
    XTR = persist.tile([128, M], F32R, name="XTR")    # [e, m] f32r
    XTB = persist.tile([128, M], BF16, name="XTB")    # [e, m] bf16
    YTR = persist.tile([128, NSLICE], F32R, name="YTR")
    nc.sync.dma_start(YTR[:], ybr)
    nc.sync.dma_start(XTR[:, 0:512], xtr[:, 0:512])
    nc.sync.dma_start(XTB[:, 0:512], xtb[:, 0:512])
    for g in range(2):
        nc.sync.dma_start(L1R[:, g, :], l1r[g])
        nc.sync.dma_start(L2R[:, g, :], l2r[g])
    nc.sync.dma_start(T1AB[:], t1ab)
    nc.sync.dma_start(T2TB[:], t2tb)
    nc.sync.dma_start(CQ[:], cq)

    # ---- persistent intermediates ---------------------------------------
    identb = const.tile([128, 128], BF16)
    make_identity(nc, identb[:])
    # dummy transposes keep the PE busy (and ramp its p-state) while the
    # first input DMAs are still in flight
    for _w in range(12):
        wp = ping([128, 128], BF16)
        nc.tensor.transpose(wp[:], identb[:], identb[:])
    RT = persist.tile([128, 2, M], F32R)       # R^T strips [32h+j, g, m]
    ST = persist.tile([128, 2, NSLICE], F32R)  # S^T strips
    # TAUG[:, mt, k, 0:16] = (X@theta1)^T-tile for head k (bf16); col 16 = 1.0
    # so the U matmul's 17th output column is the softmax denominator.
    TAUG = persist.tile([128, MT, K, 17], BF16)
    R2U = persist.tile([128, 4, K, 16], BF16)  # normalized U^T chunks
    R2T = persist.tile([128, NSLICE], BF16)    # r2^T [(k d) tight, n]
    nc.gpsimd.memset(TAUG[:, :, :, 16:17], 1.0)

    def y_block():
        ps = ping([128, 2, NSLICE])
        for g in range(2):
            nc.tensor.matmul(ps[:, g, :], lhsT=L2R[:, g, :], rhs=YTR[:],
                             start=True, stop=True)
        nc.scalar.copy(ST[:], ps[:])

    def q4_block(q4):
        # R^T group-0 projection + T tiles for m-tiles 4q..4q+3
        if q4 > 0:
            nc.sync.dma_start(XTR[:, ts(q4, 512)], xtr[:, ts(q4, 512)])
            nc.sync.dma_start(XTB[:, ts(q4, 512)], xtb[:, ts(q4, 512)])
        ps = ping([128, 512])
        nc.tensor.matmul(ps[:], lhsT=L1R[:, 0, :], rhs=XTR[:, ts(q4, 512)],
                         start=True, stop=True)
        nc.scalar.copy(RT[:, 0, ts(q4, 512)], ps[:])
        pt = ping([128, 4, K, 16])
        for j in range(4):
            mt = q4 * 4 + j
            nc.tensor.matmul(pt[:, j].rearrange("p k d -> p (k d)"),
                             lhsT=XTB[:, ts(mt, 128)], rhs=T1AB[:],
                             start=True, stop=True)
        nc.scalar.copy(TAUG[:, ds(q4 * 4, 4), :, 0:16], pt[:])

    def rs_g1_block():
        # group-1 R^T projection in two 1024-wide chunks
        for half in range(2):
            ps = ping([128, 1024])
            for c in range(2):
                nc.tensor.matmul(ps[:, ts(c, 512)], lhsT=L1R[:, 1, :],
                                 rhs=XTR[:, ts(2 * half + c, 512)],
                                 start=True, stop=True)
            nc.scalar.copy(RT[:, 1, ts(half, 1024)], ps[:])

    # ---- head pipeline: U chunk-chains drain one-per-group, one head late
    jobs = []

    def mk_uchain(U, k, c, expas):
        # one contiguous accumulation chain per n-chunk: the executor supports
        # only one open PSUM accumulation group per bank, so a chunk's chain
        # may not interleave with another chain in the same bank (complete
        # A-matmul groups in between are fine)
        def emit():
            for expa, mst, glen in expas:
                for j in range(glen):
                    mt = mst + j
                    nc.tensor.matmul(
                        U[:, c, :],
                        lhsT=expa[:, ds(j * 512 + c * 128, 128)],
                        rhs=TAUG[:, mt, k, :],
                        start=(mt == 0), stop=(mt == MT - 1))
        return emit

    def mk_finalize(U, k):
        def emit():
            rec = fin_pool.tile([128, 4], F32, tag="rec", name="rec")
            nc.vector.reciprocal(rec[:], U[:, :, 16])
            nc.vector.tensor_tensor(
                R2U[:, :, k, :], U[:, :, 0:16],
                rec[:, :, None].broadcast_to((128, 4, 16)), MULT)
        return emit

    heads_state = {}

    def head_group(k, gi):
        g, h = divmod(k, 4)
        strip = 32 * h
        if gi == 0:
            heads_state[k] = (
                ps_u.tile([128, 4, 17], F32, tag="u", name="U"), [])
        U, expas = heads_state[k]
        mst, glen = 2 * gi, 2
        aps = ping([128, 512 * glen])
        for j in range(glen):
            mt = mst + j
            nc.tensor.matmul(
                aps[:, ts(j, 512)],
                lhsT=RT[strip:strip + 16, g, ds(mt * 128, 128)],
                rhs=ST[strip:strip + 16, g, :],
                start=True, stop=True, tile_position=(strip, 0))
        expa = expa_pool.tile([128, 512 * glen], BF16, tag="e", name="expa")
        if EXP_SCHED[k][gi] == "A":
            nc.scalar.activation(expa[:], aps[:], EXP, scale=SCALE)
        else:
            nc.vector.tensor_scalar(
                expa[:].bitcast(I16), aps[:], SCH_A, SCH_B, MULT, ADD)
        expas.append((expa, mst, glen))
        # drain the previous head's jobs, one per group, spread mid-head so
        # the PE never gets a burst at the head boundary
        if gi >= 3 and len(jobs) > 5:
            jobs.pop(0)()
        if gi == NG - 1:
            for c in range(4):
                jobs.append(mk_uchain(U, k, c, expas))
            jobs.append(mk_finalize(U, k))

    # prologue interleaved with heads 0-1 (group gi needs RT chunks <= its mts)
    y_block()
    q4_block(0)
    head_group(0, 0)
    head_group(1, 0)
    q4_block(1)
    head_group(0, 1)
    head_group(1, 1)
    q4_block(2)
    head_group(0, 2)
    head_group(1, 2)
    head_group(0, 3)
    head_group(1, 3)
    q4_block(3)
    for gi in range(4, NG):
        head_group(0, gi)
        head_group(1, gi)
    for k in range(2, K):
        for gi in range(NG):
            head_group(k, gi)
            if k == 2 and gi == 0:
                rs_g1_block()
    while jobs:
        jobs.pop(0)()

    # ---- output: out^T[q, n] = theta2^T(tight).T @ r2^T + c_q ------------
    for c in range(4):
        tp = ps_u.tile([128, 128], BF16, tag="u", name="tp")
        nc.tensor.transpose(
            tp[:], R2U[:, c, :, :].rearrange("p k d -> p (k d)"), identb[:])
        nc.vector.tensor_copy(R2T[:, ts(c, 128)], tp[:])
    ot = ps_u.tile([128, NSLICE], F32, tag="u", name="ot")
    nc.tensor.matmul(ot[:], lhsT=T2TB[:], rhs=R2T[:], start=True, stop=True)
    OB = fin_pool.tile([128, NSLICE], F32, tag="ob")
    nc.vector.tensor_scalar_add(OB[:, 0:256], ot[:, 0:256], CQ[:])
    nc.sync.dma_start(outb[:, 0:256], OB[:, 0:256])
    nc.vector.tensor_scalar_add(OB[:, 256:512], ot[:, 256:512], CQ[:])
    nc.sync.dma_start(outb[:, 256:512], OB[:, 256:512])


_CACHE = {}


def build():
    if "nc" in _CACHE:
        return _CACHE["nc"]
    nc = bacc.Bacc("TRN2", target_bir_lowering=False, debug=False,
                   num_devices=NCORES)
    io = {
        "xtr": nc.dram_tensor("xtr", [E, M], F32R, kind="ExternalInput").ap(),
        "xtb": nc.dram_tensor("xtb", [E, M], BF16, kind="ExternalInput").ap(),
        "ybr": nc.dram_tensor("ybr", [E, NSLICE], F32R, kind="ExternalInput").ap(),
        "l1r": nc.dram_tensor("l1r", [2, E, 128], F32R, kind="ExternalInput").ap(),
        "l2r": nc.dram_tensor("l2r", [2, E, 128], F32R, kind="ExternalInput").ap(),
        "t1ab": nc.dram_tensor("t1ab", [E, K * D], BF16, kind="ExternalInput").ap(),
        "t2tb": nc.dram_tensor("t2tb", [K * D, E], BF16, kind="ExternalInput").ap(),
        "cq": nc.dram_tensor("cq", [E, 1], F32, kind="ExternalInput").ap(),
        "outb": nc.dram_tensor("outb", [E, NSLICE], F32, kind="ExternalOutput").ap(),
    }
    with tile.TileContext(nc) as tc:
        with ExitStack() as ctx:
            _emit(tc, ctx, io)
    nc.compile()
    _CACHE["nc"] = nc
    return nc


def make_in_maps(x, y, lambda1, lambda2, theta1, theta2, bias_lambda, bias_theta):
    import ml_dtypes
    f = np.float32
    bf = ml_dtypes.bfloat16
    l1r = np.zeros((2, E, 128), f)
    l2r = np.zeros((2, E, 128), f)
    for g in range(2):
        for h in range(4):
            l1r[g, :, 32 * h:32 * h + 16] = lambda1[4 * g + h]
            l2r[g, :, 32 * h:32 * h + 16] = lambda2[4 * g + h]
    # theta1 packed [e, (k d)]; theta2^T tight [(k d), q]; bias_theta folded
    t1ab = np.ascontiguousarray(
        theta1.transpose(1, 0, 2).reshape(E, K * D)).astype(bf)
    t2tb = np.ascontiguousarray(
        theta2.transpose(0, 2, 1).reshape(K * D, E)).astype(bf)
    cq = np.einsum("kd,kqd->q", np.asarray(bias_theta, f),
                   np.asarray(theta2, f)).reshape(E, 1).astype(f)
    xts = [np.ascontiguousarray(np.asarray(x[b], dtype=f).T) for b in range(B)]
    xtbs = [xt.astype(bf) for xt in xts]
    maps = []
    for c in range(NCORES):
        b, q = divmod(c, 4)
        maps.append({
            "xtr": xts[b],
            "xtb": xtbs[b],
            "ybr": np.ascontiguousarray(
                np.asarray(y[b, q * NSLICE:(q + 1) * NSLICE], dtype=f).T),
            "l1r": l1r, "l2r": l2r, "t1ab": t1ab, "t2tb": t2tb, "cq": cq,
        })
    return maps


def kernel(x, y, lambda1, lambda2, theta1, theta2, bias_lambda, bias_theta):
    from concourse.bass_utils import run_bass_kernel_spmd
    nc = build()
    maps = make_in_maps(x, y, lambda1, lambda2, theta1, theta2,
                        bias_lambda, bias_theta)
    res = run_bass_kernel_spmd(nc, maps, list(range(NCORES)))
    out = np.empty((B, N, E), np.float32)
    for c in range(NCORES):
        b, q = divmod(c, 4)
        out[b, q * NSLICE:(q + 1) * NSLICE] = res.results[c]["outb"].T
    return out


# revision 33
# speedup vs baseline: 1.4784x; 1.0374x over previous
"""Fused multi-head bilinear attention (softmax over query axis m) on 8 trn2 cores.

Reference computation (b=2, m=n=2048, e=128, k=8, d=16):
    r   = einsum('bmp,kpd->bmkd', x, lambda1) + bias_lambda
    A   = einsum('bmkd,kqd,bnq->kbmn', r, lambda2, y) * d**-0.5
    att = softmax(A, axis=m)
    r2  = einsum('kbmn,bmp,kpd->bnkd', att, x, theta1) + bias_theta
    out = einsum('bnkd,kqd->bnq', r2, theta2)

Sharding: 8 cores = 2 batches x 4 n-quarters (512 wide). Each core computes all 8
heads for its output slice out[b, nq*512:(nq+1)*512, :]; unshard is pure concat.

Math simplifications (exact):
  - bias_lambda adds a per-(k,n) constant to every logit column; softmax over m
    is invariant to it, so it is dropped entirely.
  - bias_theta folds into a constant output bias c_q = sum_kd bias_theta*theta2,
    precomputed on host and added during the output-PSUM evacuation.

Per-core pipeline (all heads):
  R^T/S^T strips (f32r, 2 head-groups at 32-aligned strips) and T = X@theta1
  (bf16) project from inputs that DMA straight into f32r/bf16 tiles (the host
  supplies x both ways); all loads issue upfront on HWDGE, packed into few
  transfers, with the final output half leaving over the gpsimd SWDGE path so
  the two output DMAs do not serialize.
  A tiles [m128, n512] = R^T.T @ S^T (f32r) stream in 1024-wide groups through
  THREE 2-bank PSUM pools (depth-3 software pipeline).  exp alternates per
  group between ScalarE (true exp -> bf16) and VectorE (Schraudolph bit-trick
  exp: i16 = round(A*scale*128*log2e + (16256-C)) bitcast to bf16, C=7.4,
  ~1.8% rms error that washes out over the softmax sums) so the two engines
  split the 65536-row exp load; projection evacuations ride ScalarE.  Head
  7's last two groups split each exp across both engines so the tail chain
  starts as early as possible.
  The U matmul is FLIPPED vs a streaming design: U^T[n-chunk 128, 17] +=
  expa_chunk^T @ TAUG[m, (16 cols of T | ones)] in bf16 -- 17-row matmuls
  (1 cycle/row) instead of 512-row f32r streams, cutting U from 65536 to 8704
  PE rows; the ones column makes output column 16 the softmax denominator.
  Chunk chains are emitted contiguously (the executor supports one open PSUM
  accumulation group per bank) as 5 jobs per head drained one-per-group one
  head late, so the PE never sees a head-boundary burst and never waits on a
  just-emitted exp.  Finalize = one reciprocal + one broadcast tensor_tensor
  multiply into R2U[n, (k d)] bf16 (the denominator is a native per-partition
  scalar here -- no gpsimd partition_broadcast needed).
  Tail: per 128-wide n-chunk, one full-kd PE transpose of R2U gives
  r2^T[(k d), n], one bf16 matmul against theta2^T (tight (k,d) x q packing)
  gives out^T[q, n], evacuated with the c_q bias and DMA'd out in 256-wide
  pairs as outb[E, NSLICE] (host transposes back).
"""

import sys

from contextlib import ExitStack

import numpy as np

try:
    import concourse.bass as bass
except ImportError:
    sys.path.append("/opt/trn_rl_repo")
    import concourse.bass as bass
import concourse.tile as tile
from concourse import bacc, mybir
from concourse.bass import ds, ts
from concourse.masks import make_identity

F32 = mybir.dt.float32
F32R = mybir.dt.float32r
BF16 = mybir.dt.bfloat16
I16 = mybir.dt.int16
EXP = mybir.ActivationFunctionType.Exp
MULT = mybir.AluOpType.mult
ADD = mybir.AluOpType.add

B, M, N, E, K, D = 2, 2048, 2048, 128, 8, 16
NCORES = 8
NSLICE = N // 4          # n columns per core (one batch, quarter of n)
MT = M // 128            # 16 m-tiles
SCALE = float(D) ** -0.5
LOG2E = 1.4426950408889634
SCH_C = 7.4              # Schraudolph bias tweak (exponent lsb units)
SCH_A = SCALE * 128.0 * LOG2E
SCH_B = 127.0 * 128.0 - SCH_C
NG = 8                   # A/exp groups per head, 2 m-tiles (1024 wide) each
# exp engine per group: 'D' = DVE Schraudolph, 'A' = ScalarE true exp.
# Alternation is phased so the D/A cadence stays seamless through the
# interleaved head-0/1 prologue; head 3 gives one group back to the (faster)
# ScalarE to balance total engine time (33 A / 31 D groups).
EXP_SCHED = [list("DADADADA") for _ in range(K)]
EXP_SCHED[1] = list("ADADADAD")


def _emit(tc: tile.TileContext, ctx: ExitStack, io: dict):
    nc = tc.nc
    xtr, xtb, ybr, lpr, tpb, outb = (
        io["xtr"], io["xtb"], io["ybr"], io["lpr"], io["tpb"], io["outb"],
    )

    const = ctx.enter_context(tc.tile_pool(name="const", bufs=1))
    persist = ctx.enter_context(tc.tile_pool(name="persist", bufs=1))
    # a head's expa tiles stay alive until its U jobs drain at the END of the
    # next head's groups; 3 head-generations of 8 tiles make reuse safe
    expa_pool = ctx.enter_context(tc.tile_pool(name="expa", bufs=24))
    fin_pool = ctx.enter_context(tc.tile_pool(name="fin", bufs=4))
    ps_pa = ctx.enter_context(tc.tile_pool(name="ps_pa", bufs=1, space="PSUM"))
    ps_pb = ctx.enter_context(tc.tile_pool(name="ps_pb", bufs=1, space="PSUM"))
    ps_pc = ctx.enter_context(tc.tile_pool(name="ps_pc", bufs=1, space="PSUM"))
    ps_u = ctx.enter_context(tc.tile_pool(name="ps_u", bufs=2, space="PSUM"))

    pp = [0]
    pools = [(ps_pa, "pa"), (ps_pb, "pb"), (ps_pc, "pc")]

    def ping(shape, dtype=F32):
        # strict round-robin through the three 2-bank PSUM staging pools
        pp[0] = (pp[0] + 1) % 3
        pool, tag = pools[pp[0]]
        return pool.tile(shape, dtype, tag=tag, name="pro%s" % tag)

    # ---- identity + p-state warmup first: Pool must not sit behind DMAs --
    identb = const.tile([128, 128], BF16)
    make_identity(nc, identb[:])

    # ---- input DMA: all loads issued upfront on the HWDGE path -----------
    PR = const.tile([128, 513], F32R)   # [e | L1 g0, L1 g1, L2 g0, L2 g1, cq]
    # cols 0:128 theta1 (k d) on e-rows; cols 128:256 theta2^T (tight kd x q)
    TPB = const.tile([128, 256], BF16)
    XTR = persist.tile([128, M], F32R, name="XTR")    # [e, m] f32r
    XTB = persist.tile([128, M], BF16, name="XTB")    # [e, m] bf16
    YTR = persist.tile([128, NSLICE], F32R, name="YTR")
    nc.sync.dma_start(PR[:], lpr)
    nc.sync.dma_start(XTR[:, 0:512], xtr[:, 0:512])
    nc.sync.dma_start(YTR[:], ybr)
    nc.sync.dma_start(XTB[:, 0:1024], xtb[:, 0:1024])
    nc.sync.dma_start(TPB[:], tpb)
    for q4 in range(1, 4):
        nc.sync.dma_start(XTR[:, ts(q4, 512)], xtr[:, ts(q4, 512)])
    nc.sync.dma_start(XTB[:, 1024:2048], xtb[:, 1024:2048])

    def L1R(g):
        return PR[:, ds(128 * g, 128)]

    def L2R(g):
        return PR[:, ds(256 + 128 * g, 128)]

    CQ = PR[:, 512:513].bitcast(F32)
    T1AB = TPB[:, 0:128]
    T2TB = TPB[:, 128:256]

    # dummy transposes keep the PE busy (and ramp its p-state) while the
    # first input DMAs are still in flight
    for _w in range(16):
        wp = ping([128, 128], BF16)
        nc.tensor.transpose(wp[:], identb[:], identb[:])
    RT = persist.tile([128, 2, M], F32R)       # R^T strips [32h+j, g, m]
    ST = persist.tile([128, 2, NSLICE], F32R)  # S^T strips
    # TAUG[:, mt, k, 0:16] = (X@theta1)^T-tile for head k (bf16); col 16 = 1.0
    # so the U matmul's 17th output column is the softmax denominator.
    TAUG = persist.tile([128, MT, K, 17], BF16)
    R2U = persist.tile([128, 4, K, 16], BF16)  # normalized U^T chunks
    R2TF = persist.tile([128, NSLICE], BF16)   # r2^T [(k d) tight, n]
    OBT = persist.tile([128, NSLICE], F32, name="OBT")
    nc.gpsimd.memset(TAUG[:, :, :, 16:17], 1.0)

    def y_block():
        ps = ping([128, 2, NSLICE])
        for g in range(2):
            nc.tensor.matmul(ps[:, g, :], lhsT=L2R(g), rhs=YTR[:],
                             start=True, stop=True)
        nc.scalar.copy(ST[:], ps[:])

    def rproj_block(q4):
        # R^T group-0 projection for m columns 512*q4..+512
        ps = ping([128, 512])
        nc.tensor.matmul(ps[:], lhsT=L1R(0), rhs=XTR[:, ts(q4, 512)],
                         start=True, stop=True)
        nc.scalar.copy(RT[:, 0, ts(q4, 512)], ps[:])

    def tpart_block(q4):
        # T = X@theta1 tiles for m-tiles 4q..4q+3 (bf16)
        pt = ping([128, 4, K, 16])
        for j in range(4):
            mt = q4 * 4 + j
            nc.tensor.matmul(pt[:, j].rearrange("p k d -> p (k d)"),
                             lhsT=XTB[:, ts(mt, 128)], rhs=T1AB,
                             start=True, stop=True)
        nc.scalar.copy(TAUG[:, ds(q4 * 4, 4), :, 0:16], pt[:])

    def rs_g1_block():
        # group-1 R^T projection in two 1024-wide chunks
        for half in range(2):
            ps = ping([128, 1024])
            for c in range(2):
                nc.tensor.matmul(ps[:, ts(c, 512)], lhsT=L1R(1),
                                 rhs=XTR[:, ts(2 * half + c, 512)],
                                 start=True, stop=True)
            nc.scalar.copy(RT[:, 1, ts(half, 1024)], ps[:])

    # ---- head pipeline: U chunk-chains drain one-per-group, one head late
    jobs = []

    def mk_uchain(U, k, c, expas):
        # one contiguous accumulation chain per n-chunk: the executor supports
        # only one open PSUM accumulation group per bank, so a chunk's chain
        # may not interleave with another chain in the same bank (complete
        # A-matmul groups in between are fine)
        def emit():
            for expa, mst, glen in expas:
                for j in range(glen):
                    mt = mst + j
                    nc.tensor.matmul(
                        U[:, c, :],
                        lhsT=expa[:, ds(j * 512 + c * 128, 128)],
                        rhs=TAUG[:, mt, k, :],
                        start=(mt == 0), stop=(mt == MT - 1))
        return emit

    def mk_finalize(U, k):
        def emit():
            rec = fin_pool.tile([128, 4], F32, tag="rec", name="rec")
            nc.vector.reciprocal(rec[:], U[:, :, 16])
            nc.vector.tensor_tensor(
                R2U[:, :, k, :], U[:, :, 0:16],
                rec[:, :, None].broadcast_to((128, 4, 16)), MULT)
        return emit

    def tail_chunk(c):
        # full-kd transpose of chunk c, project against theta2^T, evacuate
        # with the bias; DMA in 256-wide pairs on alternating DGE paths
        sl = ts(c, 128)
        tp = ping([128, 128], BF16)
        nc.tensor.transpose(
            tp[:], R2U[:, c, :, :].rearrange("p k d -> p (k d)"), identb[:])
        nc.vector.tensor_copy(R2TF[:, sl], tp[:])
        ot = ping([128, 128])
        nc.tensor.matmul(ot[:], lhsT=T2TB, rhs=R2TF[:, sl],
                         start=True, stop=True)
        nc.vector.tensor_scalar_add(OBT[:, sl], ot[:], CQ)
        if c == 1:
            nc.sync.dma_start(outb[:, 0:256], OBT[:, 0:256])
        elif c == 3:
            nc.gpsimd.dma_start(outb[:, 256:512], OBT[:, 256:512])

    heads_state = {}

    def head_group(k, gi):
        g, h = divmod(k, 4)
        strip = 32 * h
        if gi == 0:
            heads_state[k] = (
                ps_u.tile([128, 4, 17], F32, tag="u", name="U"), [])
        U, expas = heads_state[k]
        mst, glen = 2 * gi, 2
        aps = ping([128, 512 * glen])
        for j in range(glen):
            mt = mst + j
            nc.tensor.matmul(
                aps[:, ts(j, 512)],
                lhsT=RT[strip:strip + 16, g, ds(mt * 128, 128)],
                rhs=ST[strip:strip + 16, g, :],
                start=True, stop=True, tile_position=(strip, 0))
        expa = expa_pool.tile([128, 512 * glen], BF16, tag="e", name="expa")
        if k == K - 1 and gi >= NG - 2:
            # the tail chain hangs off the last exps: split them across both
            # engines so they finish as early as possible
            nc.scalar.activation(expa[:, 0:512], aps[:, 0:512], EXP, scale=SCALE)
            nc.vector.tensor_scalar(
                expa[:, 512:1024].bitcast(I16), aps[:, 512:1024],
                SCH_A, SCH_B, MULT, ADD)
        elif EXP_SCHED[k][gi] == "A":
            nc.scalar.activation(expa[:], aps[:], EXP, scale=SCALE)
        else:
            nc.vector.tensor_scalar(
                expa[:].bitcast(I16), aps[:], SCH_A, SCH_B, MULT, ADD)
        expas.append((expa, mst, glen))
        # drain the previous head's jobs, one per group, spread mid-head so
        # the PE never gets a burst at the head boundary
        if gi >= 3 and jobs:
            jobs.pop(0)()
        if gi == NG - 1:
            for c in range(4):
                jobs.append(mk_uchain(U, k, c, expas))
            jobs.append(mk_finalize(U, k))

    # prologue interleaved with heads 0-1 (group gi needs RT chunks <= its
    # mts); R/S projections lead so head 0's first A tile fires as soon as
    # the x/y chunks land, T projections trail behind
    rproj_block(0)
    y_block()
    head_group(0, 0)
    head_group(1, 0)
    tpart_block(0)
    rproj_block(1)
    head_group(0, 1)
    head_group(1, 1)
    tpart_block(1)
    rproj_block(2)
    head_group(0, 2)
    head_group(1, 2)
    tpart_block(2)
    head_group(0, 3)
    head_group(1, 3)
    rproj_block(3)
    tpart_block(3)
    for gi in range(4, NG):
        head_group(0, gi)
        head_group(1, gi)
    for k in range(2, K):
        for gi in range(NG):
            head_group(k, gi)
            if k == 2 and gi == 0:
                rs_g1_block()
    while jobs:
        jobs.pop(0)()
    for c in range(4):
        tail_chunk(c)


_CACHE = {}


def build():
    if "nc" in _CACHE:
        return _CACHE["nc"]
    nc = bacc.Bacc("TRN2", target_bir_lowering=False, debug=False,
                   num_devices=NCORES)
    io = {
        "xtr": nc.dram_tensor("xtr", [E, M], F32R, kind="ExternalInput").ap(),
        "xtb": nc.dram_tensor("xtb", [E, M], BF16, kind="ExternalInput").ap(),
        "ybr": nc.dram_tensor("ybr", [E, NSLICE], F32R, kind="ExternalInput").ap(),
        "lpr": nc.dram_tensor("lpr", [E, 513], F32R, kind="ExternalInput").ap(),
        "tpb": nc.dram_tensor("tpb", [E, 256], BF16, kind="ExternalInput").ap(),
        "outb": nc.dram_tensor("outb", [E, NSLICE], F32, kind="ExternalOutput").ap(),
    }
    with tile.TileContext(nc) as tc:
        with ExitStack() as ctx:
            _emit(tc, ctx, io)
    nc.compile()
    _CACHE["nc"] = nc
    return nc


def make_in_maps(x, y, lambda1, lambda2, theta1, theta2, bias_lambda, bias_theta):
    import ml_dtypes
    f = np.float32
    bf = ml_dtypes.bfloat16
    lpr = np.zeros((E, 513), f)
    for g in range(2):
        for h in range(4):
            k = 4 * g + h
            lpr[:, 128 * g + 32 * h:128 * g + 32 * h + 16] = lambda1[k]
            lpr[:, 256 + 128 * g + 32 * h:256 + 128 * g + 32 * h + 16] = lambda2[k]
    lpr[:, 512] = np.einsum("kd,kqd->q", np.asarray(bias_theta, f),
                            np.asarray(theta2, f))
    tpb = np.zeros((E, 256), bf)
    tpb[:, 0:128] = theta1.transpose(1, 0, 2).reshape(E, K * D).astype(bf)
    tpb[:, 128:256] = np.ascontiguousarray(
        theta2.transpose(0, 2, 1).reshape(K * D, E)).astype(bf)
    xts = [np.ascontiguousarray(np.asarray(x[b], dtype=f).T) for b in range(B)]
    xtbs = [xt.astype(bf) for xt in xts]
    maps = []
    for c in range(NCORES):
        b, q = divmod(c, 4)
        maps.append({
            "xtr": xts[b],
            "xtb": xtbs[b],
            "ybr": np.ascontiguousarray(
                np.asarray(y[b, q * NSLICE:(q + 1) * NSLICE], dtype=f).T),
            "lpr": lpr, "tpb": tpb,
        })
    return maps


def kernel(x, y, lambda1, lambda2, theta1, theta2, bias_lambda, bias_theta):
    from concourse.bass_utils import run_bass_kernel_spmd
    nc = build()
    maps = make_in_maps(x, y, lambda1, lambda2, theta1, theta2,
                        bias_lambda, bias_theta)
    res = run_bass_kernel_spmd(nc, maps, list(range(NCORES)))
    out = np.empty((B, N, E), np.float32)
    for c in range(NCORES):
        b, q = divmod(c, 4)
        out[b, q * NSLICE:(q + 1) * NSLICE] = res.results[c]["outb"].T
    return out
